# revision 1
# baseline (speedup 1.0000x reference)
"""Causal single-head attention (B=2, T=4096, C=1024, D=64) on 8 TRN2 cores.

Sharding: core i -> batch b = i//4, query chunk c = i%4 (rows Q0=1024c..Q0+1024).
One SPMD Bass program; per-core causal structure is entirely data-driven:
  - x[b] is rolled on host so the core's own query rows occupy key slots
    3072..4095 ("band"); the diagonal tril there is a compile-time
    affine_select, identical on every core.
  - every other key slot is fully-visible or fully-masked per core, encoded
    in a per-key additive bias folded in as a 65th contraction row of the
    S^T = K'.T@Q' matmul (row 64 of K'^T = bias, row 64 of Q'^T = 1).
  - denominator comes for free from a ones-column in V' (column 64), so the
    kernel returns unnormalized [65, 1024] = [PV^T ; rowsum]; host divides.
x is sent pre-transposed [C, T] so the kernel needs no on-device transposes
except a cheap [64,128] PE transpose of V^T -> V.
"""

import numpy as np

B, T, C, D = 2, 4096, 1024, 64
NCORES = 8
TQ = 1024          # queries per core
NKT = T // 128     # 32 key tiles of 128
BAND_KT0 = 24      # band = key tiles 24..31 (slots 3072..4095)
NEG = -1e30
DTYPE_NAME = "bfloat16"  # compute/storage dtype for x, weights, K/V/Q, P

_CACHE = {}


def _dtypes():
    import concourse.mybir as mybir
    if DTYPE_NAME == "bfloat16":
        import ml_dtypes
        return mybir.dt.bfloat16, ml_dtypes.bfloat16
    return mybir.dt.float32, np.float32


def _build_program(dt_x):
    import concourse.bass as bass
    import concourse.mybir as mybir
    import concourse.tile as tile
    from concourse import bacc
    from concourse.masks import make_identity
    from contextlib import ExitStack

    f32 = mybir.dt.float32

    nc = bacc.Bacc(
        "TRN2",
        target_bir_lowering=False,
        debug=False,
        num_devices=NCORES,
    )

    xT_t = nc.dram_tensor("xT", [C, T], dt_x, kind="ExternalInput")
    kb_t = nc.dram_tensor("kb", [1, T], dt_x, kind="ExternalInput")
    wkv_t = nc.dram_tensor("wkv", [128, 8, 128], dt_x, kind="ExternalInput")
    wq_t = nc.dram_tensor("wq", [128, 8, 64], dt_x, kind="ExternalInput")
    out_t = nc.dram_tensor("outT", [65, TQ], f32, kind="ExternalOutput")

    xT = xT_t.ap()
    kb = kb_t.ap()
    wkv = wkv_t.ap()
    wq = wq_t.ap()
    outT = out_t.ap()

    with tile.TileContext(nc) as tc, ExitStack() as ctx:
        const = ctx.enter_context(tc.tile_pool(name="const", bufs=1))
        xpool = ctx.enter_context(tc.tile_pool(name="xpool", bufs=8))
        stage = ctx.enter_context(tc.tile_pool(name="stage", bufs=3))
        ppool = ctx.enter_context(tc.tile_pool(name="ppool", bufs=3))
        psA = ctx.enter_context(tc.tile_pool(name="psA", bufs=2, space="PSUM"))
        psP = ctx.enter_context(tc.tile_pool(name="psP", bufs=2, space="PSUM"))
        psO = ctx.enter_context(tc.tile_pool(name="psO", bufs=1, space="PSUM"))

        # persistent SBUF tensors
        KT = const.tile([65, T], dt_x)        # K'^T: rows 0..63 = K^T, row 64 = key bias
        VS = const.tile([128, NKT, 65], dt_x)  # V': [:, kt, 0:64] = V rows, col 64 = 1
        QT = const.tile([65, TQ], dt_x)       # Q'^T: row 64 = 1
        wkv_sb = const.tile([128, 8, 128], dt_x)
        wq_sb = const.tile([128, 8, 64], dt_x)
        ident = const.tile([64, 64], dt_x)

        # two HWDGE queues: SP carries chunk 6 + even chunks + output,
        # Activation carries weights/bias/masks + chunk 7 + odd chunks
        nc.scalar.dma_start(out=wkv_sb, in_=wkv)
        nc.scalar.dma_start(out=wq_sb, in_=wq)
        nc.scalar.dma_start(out=KT[64:65, :], in_=kb)
        nc.vector.memset(QT[64:65, :], 1.0)
        nc.vector.memset(VS[:, :, 64:65], 1.0)
        make_identity(nc, ident)

        # band tril masks (1 where q_local >= k_local, else 0), built once
        # on otherwise-idle DVE/Pool during the DMA-bound start; applied
        # post-exp as a DVE multiply
        trilm = const.tile([128, 8, TQ], dt_x)
        nc.vector.memset(trilm, 1.0)
        for bk in range(8):
            nc.gpsimd.affine_select(
                out=trilm[:, bk, :],
                in_=trilm[:, bk, :],
                compare_op=mybir.AluOpType.is_ge,
                fill=0.0,
                base=-128 * bk,
                pattern=[[1, TQ]],
                channel_multiplier=-1,
            )

        xT_r = xT.rearrange("(a p) t -> p a t", p=128)  # [128, 8, T]

        # prefetch all 8 chunks up front (query chunks 6,7 first, per-cc
        # granularity so the first matmuls start after 1/8 of a chunk)
        xts = {}
        for tci in (6, 7, 0, 1, 2, 3, 4, 5):
            ts = slice(tci * 512, (tci + 1) * 512)
            xt = xpool.tile([128, 8, 512], dt_x, tag="xt")
            if tci >= 6:
                for cc in range(8):
                    nc.sync.dma_start(out=xt[:, cc, :], in_=xT_r[:, cc, ts])
            else:
                nc.sync.dma_start(out=xt, in_=xT_r[:, :, ts])
            xts[tci] = xt

        vts = {}

        def proj_mm(tci):
            """KV (+Q) projection matmuls for chunk tci -> KT cols, vt stage."""
            ts = slice(tci * 512, (tci + 1) * 512)
            xt = xts[tci]

            kv_ps = psP.tile([128, 512], f32, tag="pj")
            for cc in range(8):
                nc.tensor.matmul(
                    kv_ps,
                    lhsT=wkv_sb[:, cc, :],
                    rhs=xt[:, cc, :],
                    start=(cc == 0),
                    stop=(cc == 7),
                )
            if tci >= 6:
                q_ps = psP.tile([64, 512], f32, tag="pj")
                for cc in range(8):
                    nc.tensor.matmul(
                        q_ps,
                        lhsT=wq_sb[:, cc, :],
                        rhs=xt[:, cc, :],
                        start=(cc == 0),
                        stop=(cc == 7),
                    )
                qs = slice((tci - 6) * 512, (tci - 5) * 512)
                nc.vector.tensor_copy(QT[0:64, qs], q_ps)
            nc.vector.tensor_copy(KT[0:64, ts], kv_ps[0:64, :])
            vt = stage.tile([64, 512], dt_x, tag="vt")
            nc.vector.tensor_copy(vt, kv_ps[64:128, :])
            vts[tci] = vt

        def v_fixup(tci):
            """Transpose V^T chunk -> VS tiles (deferred off critical path)."""
            vt = vts.pop(tci)
            # pack 4 transposes into one PSUM bank (first sets start=True to
            # clear has_written bits; rest overwrite their own regions)
            vq = psP.tile([128, 4, 64], dt_x, tag="pj")
            for sub in range(4):
                nc.tensor.matmul(
                    vq[:, sub, :],
                    lhsT=vt[:, sub * 128:(sub + 1) * 128],
                    rhs=ident,
                    is_transpose=True,
                    start=(sub == 0),
                    stop=(sub == 3),
                    skip_group_check=True,
                )
            nc.vector.tensor_copy(VS[:, tci * 4:tci * 4 + 4, 0:64], vq)

        def proj_pieces(tci):
            """proj_chunk as a list of small closures, to smear across the
            attend pipeline so PE load stays even."""
            ts = slice(tci * 512, (tci + 1) * 512)
            xt = xts[tci]
            kv_ps = psP.tile([128, 512], f32, tag="pj")

            def mk(cc0):
                def f():
                    for cc in (cc0, cc0 + 1):
                        nc.tensor.matmul(
                            kv_ps,
                            lhsT=wkv_sb[:, cc, :],
                            rhs=xt[:, cc, :],
                            start=(cc == 0),
                            stop=(cc == 7),
                        )
                return f

            def finish():
                qs = slice(tci * 512, (tci + 1) * 512)
                nc.vector.tensor_copy(KT[0:64, qs], kv_ps[0:64, :])
                vt = stage.tile([64, 512], dt_x, tag="vt")
                nc.vector.tensor_copy(vt, kv_ps[64:128, :])
                vts[tci] = vt

            return [mk(0), mk(2), mk(4), mk(6), finish,
                    lambda: v_fixup(tci)]

        pv = psO.tile([65, TQ], f32)

        def attend_S(kt):
            """S^T matmuls for one key tile (PE)."""
            s_ps = psA.tile([128, TQ], f32, tag="s")
            for qh in range(2):
                qs = slice(qh * 512, (qh + 1) * 512)
                nc.tensor.matmul(
                    s_ps[:, qs],
                    lhsT=KT[:, kt * 128:(kt + 1) * 128],
                    rhs=QT[:, qs],
                    start=True,
                    stop=True,
                )
            return s_ps

        def attend_rest(kt, s_ps, first, last):
            """exp -> (band tril) -> PV accumulate for one key tile."""
            p_sb = ppool.tile([128, TQ], dt_x, tag="p")
            nc.scalar.activation(
                p_sb, s_ps, mybir.ActivationFunctionType.Exp, scale=float(D) ** -0.5
            )
            if kt >= BAND_KT0:
                # band tril: zero P above the diagonal (prebuilt 0/1 mask)
                nc.vector.tensor_mul(p_sb, p_sb, trilm[:, kt - BAND_KT0, :])
            for qh in range(2):
                qs = slice(qh * 512, (qh + 1) * 512)
                nc.tensor.matmul(
                    pv[:, qs],
                    lhsT=VS[:, kt, :],
                    rhs=p_sb[:, qs],
                    start=first,
                    stop=last,
                )

        # Software-pipelined, interleaved schedule: query chunks (6,7)
        # projected first so band key-tiles can attend immediately; the
        # S^T matmul of kt+1 is emitted before PV of kt so PE never stalls
        # on ACT's exp; remaining projections fill PE gaps.
        proj_mm(6)
        proj_mm(7)
        v_fixup(6)
        v_fixup(7)
        order = list(range(BAND_KT0, NKT)) + list(range(BAND_KT0))

        pending = []   # [(tci, closure)] proj pieces smeared across attends
        queued = set()

        def queue_chunk(c):
            if c in queued or not (0 <= c <= 5):
                return
            queued.add(c)
            pending.extend((c, f) for f in proj_pieces(c))

        def drain_chunk(c):
            rest = []
            for tc, f in pending:
                if tc == c:
                    f()
                else:
                    rest.append((tc, f))
            pending[:] = rest

        queue_chunk(0)
        queue_chunk(1)
        pipe = []  # [(kt, s_ps)]
        for kt in order:
            if kt < BAND_KT0 and kt % 4 == 0:
                drain_chunk(kt // 4)
                queue_chunk(kt // 4 + 2)
            pipe.append((kt, attend_S(kt)))
            if len(pipe) > 1:
                pkt, ps = pipe.pop(0)
                attend_rest(pkt, ps, first=(pkt == order[0]), last=False)
            for _ in range(2):
                if pending:
                    tc, f = pending.pop(0)
                    f()
        pkt, ps = pipe.pop(0)
        attend_rest(pkt, ps, first=False, last=True)

        osb = stage.tile([65, TQ], f32, tag="o")
        for qh in range(2):  # halves so copy/DMA overlap the last PV matmul
            qs = slice(qh * 512, (qh + 1) * 512)
            nc.vector.tensor_copy(osb[:, qs], pv[:, qs])
            nc.sync.dma_start(out=outT[:, qs], in_=osb[:, qs])

    nc.compile()
    return nc


def _prep_inputs(x, Wq, Wk, Wv, np_dt):
    """Per-core input maps."""
    wkv = np.empty((128, 8, 128), dtype=np_dt)
    wkv[:, :, 0:64] = Wk.reshape(8, 128, 64).transpose(1, 0, 2)
    wkv[:, :, 64:128] = Wv.reshape(8, 128, 64).transpose(1, 0, 2)
    wq = np.ascontiguousarray(
        Wq.reshape(8, 128, 64).transpose(1, 0, 2)).astype(np_dt)

    # band tril masks: trilm[p, bk, q] = 1.0 if q >= 128*bk + p else 0.0
    q_idx = np.arange(TQ)
    trilm = np.empty((128, 8, TQ), dtype=np_dt)
    for bk in range(8):
        for p_row in range(128):
            trilm[p_row, bk, :] = (q_idx >= 128 * bk + p_row)

    in_maps = []
    for core in range(NCORES):
        b, c = divmod(core, 4)
        Q0 = TQ * c
        xr = np.roll(x[b], -(Q0 + TQ), axis=0)  # slot s -> abs key (s+Q0+TQ)%T
        xT = np.ascontiguousarray(xr.T).astype(np_dt)
        absk = (np.arange(T) + Q0 + TQ) % T
        kbias = np.where(absk < Q0, 0.0, NEG).astype(np_dt)
        kbias[T - TQ:] = 0.0  # band slots: tril handled on-device
        in_maps.append({
            "xT": xT,
            "kb": kbias.reshape(1, T),
            "wkv": wkv,
            "wq": wq,
        })
    return in_maps


def kernel(x, Wq, Wk, Wv, _trace=False):
    from concourse.bass_utils import run_bass_kernel_spmd

    dt_x, np_dt = _dtypes()

    key = ("prog", str(dt_x))
    if key not in _CACHE:
        _CACHE[key] = _build_program(dt_x)
    nc = _CACHE[key]

    in_maps = _prep_inputs(
        np.asarray(x, np.float32), np.asarray(Wq, np.float32),
        np.asarray(Wk, np.float32), np.asarray(Wv, np.float32), np_dt)

    res = run_bass_kernel_spmd(
        nc, in_maps, core_ids=list(range(NCORES)), trace=_trace)

    out = np.empty((B, T, D), dtype=np.float32)
    for core in range(NCORES):
        b, c = divmod(core, 4)
        o = res.results[core]["outT"]  # [65, TQ]
        out[b, TQ * c:TQ * (c + 1), :] = (o[0:64, :] / o[64:65, :]).T
    if _trace:
        return out, res
    return out



# revision 2
# speedup vs baseline: 607.5523x; 607.5523x over previous
"""Causal single-head attention (B=2, T=4096, C=1024, D=64) on 8 TRN2 cores.

Sharding: core i -> batch b = i//4, query phase c = i%4: the core owns the
strided query rows {4j + c : j in [0,1024)}. This balances causal work
exactly across cores AND lets each core skip fully-masked key tiles:

  - x[b] is column-permuted on host (within every group of 4 columns,
    rotate by c) so the core's query columns sit at slots 4j — a
    compile-time stride-4 slice, identical on every core.
  - key tile kt (slots 128kt..128kt+127) is attended only by query
    columns j >= 32kt ("suffix" structure): columns j >= 32(kt+1) are
    fully visible, j in [32kt, 32kt+32) are the diagonal band (one
    host-computed [128,32] 0/1 mask, identical for all kt), and
    j < 32kt are fully masked — never computed.
  - denominator comes free from a ones-column in V' (column 64); the
    kernel returns unnormalized [65, 1024] = [PV^T ; rowsum]; host divides.

Key tiles are processed in DESCENDING order so attention starts as soon
as the LAST x chunk arrives (chunks DMA'd in reverse): chunk ch supplies
both key tiles 4ch..4ch+3 and query columns [128ch, 128(ch+1)), and key
tile kt only needs query columns [32kt, 1024) — exactly what's loaded.
PV accumulates into suffix ranges of a pre-zeroed PSUM bank (dummy
zero matmul opens the accumulation group).
"""

import numpy as np

B, T, C, D = 2, 4096, 1024, 64
NCORES = 8
TQ = 1024          # queries per core (strided by 4)
NKT = T // 128     # 32 key tiles of 128
DTYPE_NAME = "bfloat16"  # compute/storage dtype for x, weights, K/V/Q, P

_CACHE = {}


def _dtypes():
    import concourse.mybir as mybir
    if DTYPE_NAME == "bfloat16":
        import ml_dtypes
        return mybir.dt.bfloat16, ml_dtypes.bfloat16
    return mybir.dt.float32, np.float32


def _build_program(dt_x):
    import concourse.bass as bass
    import concourse.mybir as mybir
    import concourse.tile as tile
    from concourse import bacc
    from concourse.masks import make_identity
    from contextlib import ExitStack

    f32 = mybir.dt.float32

    nc = bacc.Bacc(
        "TRN2",
        target_bir_lowering=False,
        debug=False,
        num_devices=NCORES,
    )

    xT_t = nc.dram_tensor("xT", [C, T], dt_x, kind="ExternalInput")
    wkv_t = nc.dram_tensor("wkv", [128, 8, 128], dt_x, kind="ExternalInput")
    wq_t = nc.dram_tensor("wq", [128, 8, 64], dt_x, kind="ExternalInput")
    m32_t = nc.dram_tensor("m32", [128, 32], dt_x, kind="ExternalInput")
    out_t = nc.dram_tensor("outT", [65, TQ], f32, kind="ExternalOutput")

    xT = xT_t.ap()
    wkv = wkv_t.ap()
    wq = wq_t.ap()
    m32 = m32_t.ap()
    outT = out_t.ap()

    with tile.TileContext(nc) as tc, ExitStack() as ctx:
        const = ctx.enter_context(tc.tile_pool(name="const", bufs=1))
        xpool = ctx.enter_context(tc.tile_pool(name="xpool", bufs=8))
        stage = ctx.enter_context(tc.tile_pool(name="stage", bufs=3))
        ppool = ctx.enter_context(tc.tile_pool(name="ppool", bufs=3))
        psA = ctx.enter_context(tc.tile_pool(name="psA", bufs=2, space="PSUM"))
        psP = ctx.enter_context(tc.tile_pool(name="psP", bufs=2, space="PSUM"))
        psO = ctx.enter_context(tc.tile_pool(name="psO", bufs=1, space="PSUM"))

        # persistent SBUF tensors
        KT = const.tile([64, T], dt_x)         # K^T, key slot order
        VS = const.tile([128, NKT, 65], dt_x)  # V': [:, kt, 0:64] = V rows, col 64 = 1
        QT = const.tile([64, TQ], dt_x)        # Q^T, local query cols
        wkv_sb = const.tile([128, 8, 128], dt_x)
        wq_sb = const.tile([128, 8, 64], dt_x)
        m32_sb = const.tile([128, 32], dt_x)
        ident = const.tile([64, 64], dt_x)
        zl = const.tile([1, 65], dt_x)         # zeros for PSUM-clearing matmul
        zr = const.tile([1, 512], dt_x)

        nc.scalar.dma_start(out=wkv_sb, in_=wkv)
        nc.scalar.dma_start(out=wq_sb, in_=wq)
        nc.scalar.dma_start(out=m32_sb, in_=m32)
        nc.vector.memset(VS[:, :, 64:65], 1.0)
        nc.vector.memset(zl, 0.0)
        nc.vector.memset(zr, 0.0)
        make_identity(nc, ident)

        xT_r = xT.rearrange("(a p) t -> p a t", p=128)  # [128, 8, T]

        # prefetch chunks in reverse (attention consumes kt descending)
        xts = {}
        for tci in range(7, -1, -1):
            ts = slice(tci * 512, (tci + 1) * 512)
            xt = xpool.tile([128, 8, 512], dt_x, tag="xt")
            nc.sync.dma_start(out=xt, in_=xT_r[:, :, ts])
            xts[tci] = xt

        pv = psO.tile([65, TQ], f32)
        # open the accumulation group with a zeroing matmul (contraction-1
        # outer product of zeros) so all PV matmuls can accumulate
        for h in range(2):
            nc.tensor.matmul(
                pv[:, h * 512:(h + 1) * 512],
                lhsT=zl, rhs=zr,
                start=True, stop=False, skip_group_check=True,
            )

        def proj_chunk(tci):
            """K/V/Q projections + V transpose for chunk tci."""
            ts = slice(tci * 512, (tci + 1) * 512)
            xt = xts[tci]
            kv_ps = psP.tile([128, 512], f32, tag="pj")
            for cc in range(8):
                nc.tensor.matmul(
                    kv_ps,
                    lhsT=wkv_sb[:, cc, :],
                    rhs=xt[:, cc, :],
                    start=(cc == 0),
                    stop=(cc == 7),
                )
            nc.vector.tensor_copy(KT[0:64, ts], kv_ps[0:64, :])
            vt = stage.tile([64, 512], dt_x, tag="vt")
            nc.vector.tensor_copy(vt, kv_ps[64:128, :])

            # query columns of this chunk: slots 4f -> local query 128*tci+f
            xt4 = xt.rearrange("p a (f g) -> p a f g", g=4)
            qx = stage.tile([128, 8, 128], dt_x, tag="qx")
            nc.vector.tensor_copy(qx, xt4[:, :, :, 0])
            q_ps = psP.tile([64, 128], f32, tag="pj")
            for cc in range(8):
                nc.tensor.matmul(
                    q_ps,
                    lhsT=wq_sb[:, cc, :],
                    rhs=qx[:, cc, :],
                    start=(cc == 0),
                    stop=(cc == 7),
                )
            nc.vector.tensor_copy(QT[0:64, 128 * tci:128 * (tci + 1)], q_ps)

            # V^T chunk -> VS tiles (4 PE transposes packed into one bank)
            vq = psP.tile([128, 4, 64], dt_x, tag="pj")
            for sub in range(4):
                nc.tensor.matmul(
                    vq[:, sub, :],
                    lhsT=vt[:, sub * 128:(sub + 1) * 128],
                    rhs=ident,
                    is_transpose=True,
                    start=(sub == 0),
                    stop=(sub == 3),
                    skip_group_check=True,
                )
            nc.vector.tensor_copy(VS[:, tci * 4:tci * 4 + 4, 0:64], vq)

        def attend_S(kt):
            """S^T suffix matmuls for one key tile (PE)."""
            qlo = 32 * kt
            s_ps = psA.tile([128, TQ], f32, tag="s")
            spans = [(qlo, 512), (512, TQ)] if qlo < 512 else [(qlo, TQ)]
            for lo, hi in spans:
                nc.tensor.matmul(
                    s_ps[:, lo:hi],
                    lhsT=KT[:, kt * 128:(kt + 1) * 128],
                    rhs=QT[:, lo:hi],
                    start=True,
                    stop=True,
                )
            return s_ps

        def attend_rest(kt, s_ps, last):
            """exp -> band mask -> PV accumulate for one key tile."""
            qlo = 32 * kt
            p_sb = ppool.tile([128, TQ], dt_x, tag="p")
            nc.scalar.activation(
                p_sb[:, qlo:], s_ps[:, qlo:],
                mybir.ActivationFunctionType.Exp, scale=float(D) ** -0.5,
            )
            nc.vector.tensor_mul(
                p_sb[:, qlo:qlo + 32], p_sb[:, qlo:qlo + 32], m32_sb)
            spans = [(qlo, 512), (512, TQ)] if qlo < 512 else [(qlo, TQ)]
            for lo, hi in spans:
                nc.tensor.matmul(
                    pv[:, lo:hi],
                    lhsT=VS[:, kt, :],
                    rhs=p_sb[:, lo:hi],
                    start=False,
                    stop=last,
                    skip_group_check=True,
                )

        # pipeline: S(kt) runs ahead of exp/PV(kt); next chunk's projections
        # are emitted right after the first S of the current chunk so PE has
        # projection work while ACT runs exp.
        proj_chunk(7)
        pipe = []  # [(kt, s_ps)]
        for kt in range(NKT - 1, -1, -1):
            pipe.append((kt, attend_S(kt)))
            if kt % 4 == 3 and kt >= 4:
                proj_chunk(kt // 4 - 1)
            if len(pipe) > 1:
                pkt, ps = pipe.pop(0)
                attend_rest(pkt, ps, last=False)
        pkt, ps = pipe.pop(0)
        attend_rest(pkt, ps, last=True)

        osb = stage.tile([65, TQ], f32, tag="o")
        for qh in range(2):  # halves so copy/DMA overlap the last PV matmul
            qs = slice(qh * 512, (qh + 1) * 512)
            nc.vector.tensor_copy(osb[:, qs], pv[:, qs])
            nc.sync.dma_start(out=outT[:, qs], in_=osb[:, qs])

    nc.compile()
    return nc


def _prep_inputs(x, Wq, Wk, Wv, np_dt):
    """Per-core input maps."""
    wkv = np.empty((128, 8, 128), dtype=np_dt)
    wkv[:, :, 0:64] = Wk.reshape(8, 128, 64).transpose(1, 0, 2)
    wkv[:, :, 64:128] = Wv.reshape(8, 128, 64).transpose(1, 0, 2)
    wq = np.ascontiguousarray(
        Wq.reshape(8, 128, 64).transpose(1, 0, 2)).astype(np_dt)

    s = np.arange(T)
    p_idx = np.arange(128)[:, None]
    col = np.arange(32)[None, :]

    in_maps = []
    for core in range(NCORES):
        b, c = divmod(core, 4)
        # column roll: slot s <- abs column 4*(s//4) + ((s%4 + c) % 4)
        perm = 4 * (s // 4) + ((s % 4 + c) % 4)
        xT = np.ascontiguousarray(x[b].T[:, perm]).astype(np_dt)
        # band mask: key slot p (of its tile) visible to band column col?
        abs_k = 4 * (p_idx // 4) + ((p_idx % 4 + c) % 4)
        abs_q = 4 * col + c
        m32 = (abs_k <= abs_q).astype(np_dt)
        in_maps.append({
            "xT": xT,
            "wkv": wkv,
            "wq": wq,
            "m32": m32,
        })
    return in_maps


def kernel(x, Wq, Wk, Wv, _trace=False, _trace_cores=None):
    from concourse.bass_utils import run_bass_kernel_spmd

    dt_x, np_dt = _dtypes()

    key = ("prog", str(dt_x))
    if key not in _CACHE:
        _CACHE[key] = _build_program(dt_x)
    nc = _CACHE[key]

    in_maps = _prep_inputs(
        np.asarray(x, np.float32), np.asarray(Wq, np.float32),
        np.asarray(Wk, np.float32), np.asarray(Wv, np.float32), np_dt)

    res = run_bass_kernel_spmd(
        nc, in_maps, core_ids=list(range(NCORES)), trace=_trace,
        trace_cores=_trace_cores)

    jidx = 4 * np.arange(TQ)
    out = np.empty((B, T, D), dtype=np.float32)
    for core in range(NCORES):
        b, c = divmod(core, 4)
        o = res.results[core]["outT"]  # [65, TQ]
        out[b, jidx + c, :] = (o[0:64, :] / o[64:65, :]).T
    if _trace:
        return out, res
    return out
